# revision 6
# baseline (speedup 1.0000x reference)
"""Trainium2 Bass kernel for nn_DiceLoss (soft dice, binary task, average='batch',
channel_reduction='mean').

    num_c = 2 * sum_{b,n} x[b,c,n] * t[b,c,n]
    den_c = sum x^2 + sum t^2
    out   = 1 - mean_c (num_c + eps) / (den_c + eps)

Strategy (data-parallel over batch, 8 NeuronCores):
  - core i gets batches [2i, 2i+2) of both tensors, rearranged host-side to
    channel-major [3, 128, 4096] so each channel is contiguous
  - per channel, split into 2 chunks; per chunk one DMA pair loads the x and t
    halves of a combined [128, 4096] SBUF tile (1 MiB per DMA); VectorE
    computes sum(x*t) per partition via a fused scalar_tensor_tensor
    (mult+mult, accum_out); ScalarE computes sum(x^2)+sum(t^2) per partition
    via one activation(Square, accum_out) over the combined tile (the dice
    denominator only needs the sum, so x^2 and t^2 need not be separated)
  - per-core per-partition partials ([128, 6] per core) are summed on host and
    fed through the (tiny) dice formula in float64

The kernel is memory-bound: 12.6 MiB HBM reads per core (~35 us at the
~358 GB/s per-core HBM limit); VectorE (~13 us) and ScalarE (~21 us) hide
under the DMA.  TimelineSim (production cost model): ~45 us/core.

Self-contained: hardcodes shapes from the problem spec
(input/target: [16, 3, 512, 512] float32).
"""

import numpy as np

import concourse.bass as bass
import concourse.tile as tile
from concourse import mybir
from concourse.bass_utils import run_bass_kernel_spmd

EPS = 1e-05

B, C, H, W = 16, 3, 512, 512
N_CORES = 8
B_PER_CORE = B // N_CORES            # 2
SPATIAL = H * W                      # 262144
CH = B_PER_CORE * SPATIAL            # 524288 elems per channel per core
P = 128
F = CH // P                          # 4096
# chunk schedule (free-dim columns per chunk, per channel): 1 MiB DMAs except
# a tapered final channel so the last ScalarE op on the critical tail is small
SPLITS = [[2048, 2048], [2048, 2048], [2048, 1024, 1024]]
N_ACCS = sum(len(s) for s in SPLITS)


# --- walrus workaround: this container's neuronxcc rejects instructions with
# more than one sync-wait; hoist excess waits onto same-engine NOPs. ---
_uid = [0]


def _split_multi_waits(nc):
    for fn in nc.m.functions:
        for bb in fn.blocks:
            insts = bb.instructions
            out = []
            changed = False
            for inst in insts:
                si = inst.sync_info
                waits = list(si.on_wait) if si is not None and si.on_wait else []
                if len(waits) > 1:
                    changed = True
                    for w in waits[:-1]:
                        _uid[0] += 1
                        out.append(mybir.InstNoOp(
                            name=f"waitsplit_{_uid[0]}",
                            engine=inst.engine,
                            bass_nofuse=True,
                            sync_info=mybir.SyncInfo(on_wait=[w], on_update=[]),
                        ))
                    si.on_wait = [waits[-1]]
                out.append(inst)
            if changed:
                bb.instructions = out


def _trimmed_drain_and_barrier(self, tick_clock, wait_clock):
    """Tile kernel tail without the second all-engine barrier.

    Default tail: drain(+waits) / barrier / semaphore clear / barrier.  After
    the first barrier every engine is past its last real instruction, and only
    GpSimd runs the dma_reset+sem_clear; the runtime does not report the NEFF
    done until GpSimd's queue (including the clear) retires, so the trailing
    barrier only adds EVSEM-butterfly latency (~us on silicon) and is dropped.
    (The drain's multi-wait list is split by _split_multi_waits afterwards.)
    """
    from concourse.tile import ScopedClock

    nc = self.nc
    drain_inst = nc.sync.drain()
    wait_clock.add_sem_waits(
        drain_inst.ins, ScopedClock({None: tick_clock.global_clock})
    )
    nc.all_engine_barrier()
    assert self.sems is not None
    popped = nc._tile_sem_poison_stack.pop()
    assert popped is self._sem_poison
    nc.clear_and_free_semaphores(list(self.sems.allocated().values()))


def build_dice_program(repeats: int = 1):
    """Bass program: inputs x,t [3, 128, 4096] f32; output stats [128, 6] f32.

    stats[:, c]   = per-partition sum of x*t for channel c
    stats[:, 3+c] = per-partition sum of x^2 + t^2 for channel c

    repeats > 1 re-runs the whole body (for marginal-time benchmarking);
    only the last repeat's stats land in the output.
    """
    from contextlib import ExitStack

    nc = bass.Bass("TRN2", target_bir_lowering=False, debug=False,
                   num_devices=N_CORES)
    x_d = nc.dram_tensor("x", [C, P, F], mybir.dt.float32,
                         kind="ExternalInput").ap()
    t_d = nc.dram_tensor("t", [C, P, F], mybir.dt.float32,
                         kind="ExternalInput").ap()
    out_d = nc.dram_tensor("stats", [P, 2 * C], mybir.dt.float32,
                           kind="ExternalOutput").ap()

    orig_tail = tile.TileContext._drain_and_barrier
    tile.TileContext._drain_and_barrier = _trimmed_drain_and_barrier
    try:
        _build_body(nc, x_d, t_d, out_d, repeats)
    finally:
        tile.TileContext._drain_and_barrier = orig_tail

    _split_multi_waits(nc)
    return nc


def _build_body(nc, x_d, t_d, out_d, repeats):
    from contextlib import ExitStack

    with tile.TileContext(nc) as tc, ExitStack() as ctx:
        inp = ctx.enter_context(tc.tile_pool(name="inp", bufs=4))
        dve_scr = ctx.enter_context(tc.tile_pool(name="dve_scr", bufs=2))
        act_scr = ctx.enter_context(tc.tile_pool(name="act_scr", bufs=2))
        accp = ctx.enter_context(
            tc.tile_pool(name="acc", bufs=N_ACCS + 2))
        outp = ctx.enter_context(tc.tile_pool(name="outp", bufs=1))

        for r in range(repeats):
            accs = {c: [] for c in range(C)}
            for c in range(C):
                off = 0
                for FS in SPLITS[c]:
                    xt = inp.tile([P, 2 * FS], mybir.dt.float32, tag="xt")
                    nc.sync.dma_start(out=xt[:, 0:FS],
                                      in_=x_d[c, :, off:off + FS])
                    nc.sync.dma_start(out=xt[:, FS:2 * FS],
                                      in_=t_d[c, :, off:off + FS])
                    off += FS

                    acc_num = accp.tile([P, 1], mybir.dt.float32,
                                        tag="acc_num")
                    acc_den = accp.tile([P, 1], mybir.dt.float32,
                                        tag="acc_den")

                    prod = dve_scr.tile([P, FS], mybir.dt.float32, tag="prod")
                    nc.vector.scalar_tensor_tensor(
                        out=prod[:], in0=xt[:, 0:FS], scalar=1.0,
                        in1=xt[:, FS:2 * FS],
                        op0=mybir.AluOpType.mult, op1=mybir.AluOpType.mult,
                        accum_out=acc_num[:],
                    )
                    sq = act_scr.tile([P, 2 * FS], mybir.dt.float32, tag="sq")
                    nc.scalar.activation(
                        out=sq[:], in_=xt[:],
                        func=mybir.ActivationFunctionType.Square,
                        accum_out=acc_den[:],
                    )
                    accs[c].append((acc_num, acc_den))

            if r == repeats - 1:
                gather = outp.tile([P, 2 * C], mybir.dt.float32)
                for c in range(C):
                    (num0, den0) = accs[c][0]
                    for (num, den) in accs[c][1:]:
                        nc.vector.tensor_add(num0[:], num0[:], num[:])
                        nc.vector.tensor_add(den0[:], den0[:], den[:])
                    nc.vector.tensor_copy(gather[:, c:c + 1], num0[:])
                    nc.vector.tensor_copy(gather[:, C + c:C + c + 1], den0[:])
                nc.sync.dma_start(out=out_d[:], in_=gather[:])


def _shard_inputs(input, target):
    """Rearrange full [16,3,512,512] arrays into 8 per-core channel-major
    [3, 128, 4096] float32 arrays."""
    in_maps = []
    for i in range(N_CORES):
        maps = {}
        for name, arr in (("x", input), ("t", target)):
            shard = arr[i * B_PER_CORE:(i + 1) * B_PER_CORE]  # [2,3,512,512]
            shard = np.ascontiguousarray(
                np.transpose(shard, (1, 0, 2, 3))
            ).reshape(C, P, F)
            maps[name] = shard
        in_maps.append(maps)
    return in_maps


def _finalize(stats_per_core):
    """stats_per_core: list of [128, 6] arrays -> scalar dice loss."""
    total = np.zeros(2 * C, dtype=np.float64)
    for s in stats_per_core:
        total += s.astype(np.float64).sum(axis=0)
    num = 2.0 * total[:C]
    den = total[C:]
    dice = (num + EPS) / (den + EPS)
    return np.array(1.0 - dice.mean(), dtype=np.float32)


_CACHED = {}


def _get_program():
    if "nc" not in _CACHED:
        _CACHED["nc"] = build_dice_program(repeats=1)
    return _CACHED["nc"]


def kernel(input: np.ndarray, target: np.ndarray) -> np.ndarray:
    input = np.asarray(input, dtype=np.float32)
    target = np.asarray(target, dtype=np.float32)
    assert input.shape == (B, C, H, W) and target.shape == (B, C, H, W)

    nc = _get_program()
    in_maps = _shard_inputs(input, target)
    res = run_bass_kernel_spmd(nc, in_maps, core_ids=list(range(N_CORES)))
    return _finalize([res.results[i]["stats"] for i in range(N_CORES)])


# revision 7
# speedup vs baseline: 1.2160x; 1.2160x over previous
"""Trainium2 Bass kernel for nn_DiceLoss (soft dice, binary task, average='batch',
channel_reduction='mean').

    num_c = 2 * sum_{b,n} x[b,c,n] * t[b,c,n]
    den_c = sum x^2 + sum t^2
    out   = 1 - mean_c (num_c + eps) / (den_c + eps)

Strategy (data-parallel over batch, 8 NeuronCores):
  - core i gets batches [2i, 2i+2) of both tensors, rearranged host-side to
    channel-major [3, 128, 4096] so each channel is contiguous
  - per channel, chunked DMA pairs load the x and t halves of a combined
    [128, 2*FS] SBUF tile (1 MiB per DMA, final channel tapered so the last
    ScalarE op on the critical tail is small); VectorE
    computes sum(x*t) per partition via a fused scalar_tensor_tensor
    (mult+mult, accum_out); ScalarE computes sum(x^2)+sum(t^2) per partition
    via one activation(Square, accum_out) over the combined tile (the dice
    denominator only needs the sum, so x^2 and t^2 need not be separated)
  - per-core per-partition partials ([128, 6] per core) are summed on host and
    fed through the (tiny) dice formula in float64

The kernel is memory-bound: 12.6 MiB HBM reads per core (~35 us at the
~358 GB/s per-core HBM limit); VectorE (~13 us) and ScalarE (~21 us) hide
under the DMA.  TimelineSim (production cost model): ~43 us/core.

Self-contained: hardcodes shapes from the problem spec
(input/target: [16, 3, 512, 512] float32).
"""

import numpy as np

import concourse.bass as bass
import concourse.tile as tile
from concourse import mybir
from concourse.bass_utils import run_bass_kernel_spmd

EPS = 1e-05

B, C, H, W = 16, 3, 512, 512
N_CORES = 8
B_PER_CORE = B // N_CORES            # 2
SPATIAL = H * W                      # 262144
CH = B_PER_CORE * SPATIAL            # 524288 elems per channel per core
P = 128
F = CH // P                          # 4096
# chunk schedule (free-dim columns per chunk, per channel): 1 MiB DMAs except
# a tapered final channel so the last ScalarE op on the critical tail is small
SPLITS = [[2048, 2048], [2048, 2048], [2048, 1024, 1024]]
N_ACCS = sum(len(s) for s in SPLITS)


# --- walrus workaround: this container's neuronxcc rejects instructions with
# more than one sync-wait; hoist excess waits onto same-engine NOPs. ---
_uid = [0]


def _split_multi_waits(nc):
    for fn in nc.m.functions:
        for bb in fn.blocks:
            insts = bb.instructions
            out = []
            changed = False
            for inst in insts:
                si = inst.sync_info
                waits = list(si.on_wait) if si is not None and si.on_wait else []
                if len(waits) > 1:
                    changed = True
                    for w in waits[:-1]:
                        _uid[0] += 1
                        out.append(mybir.InstNoOp(
                            name=f"waitsplit_{_uid[0]}",
                            engine=inst.engine,
                            bass_nofuse=True,
                            sync_info=mybir.SyncInfo(on_wait=[w], on_update=[]),
                        ))
                    si.on_wait = [waits[-1]]
                out.append(inst)
            if changed:
                bb.instructions = out


def _trimmed_drain_and_barrier(self, tick_clock, wait_clock):
    """Tile kernel tail without the trailing all-engine barrier.

    Default tail: drain(+waits) / barrier / semaphore clear / barrier.  After
    the first barrier every engine is past its last real instruction, and only
    GpSimd runs the dma_reset+sem_clear; the runtime does not report the NEFF
    done until GpSimd's queue (including the clear) retires, so the trailing
    barrier only adds EVSEM-butterfly latency (~us on silicon) and is dropped.
    (The drain's multi-wait list is split by _split_multi_waits afterwards.)
    """
    from concourse.tile import ScopedClock

    nc = self.nc
    drain_inst = nc.sync.drain()
    wait_clock.add_sem_waits(
        drain_inst.ins, ScopedClock({None: tick_clock.global_clock})
    )
    nc.all_engine_barrier()
    assert self.sems is not None
    popped = nc._tile_sem_poison_stack.pop()
    assert popped is self._sem_poison
    nc.clear_and_free_semaphores(list(self.sems.allocated().values()))


def build_dice_program(repeats: int = 1):
    """Bass program: inputs x,t [3, 128, 4096] f32; output stats [128, 6] f32.

    stats[:, c]   = per-partition sum of x*t for channel c
    stats[:, 3+c] = per-partition sum of x^2 + t^2 for channel c

    repeats > 1 re-runs the whole body (for marginal-time benchmarking);
    only the last repeat's stats land in the output.
    """
    from contextlib import ExitStack

    nc = bass.Bass("TRN2", target_bir_lowering=False, debug=False,
                   num_devices=N_CORES)
    x_d = nc.dram_tensor("x", [C, P, F], mybir.dt.float32,
                         kind="ExternalInput").ap()
    t_d = nc.dram_tensor("t", [C, P, F], mybir.dt.float32,
                         kind="ExternalInput").ap()
    out_d = nc.dram_tensor("stats", [P, 2 * C], mybir.dt.float32,
                           kind="ExternalOutput").ap()

    orig_tail = tile.TileContext._drain_and_barrier
    tile.TileContext._drain_and_barrier = _trimmed_drain_and_barrier
    try:
        _build_body(nc, x_d, t_d, out_d, repeats)
    finally:
        tile.TileContext._drain_and_barrier = orig_tail

    _split_multi_waits(nc)
    return nc


def _build_body(nc, x_d, t_d, out_d, repeats):
    from contextlib import ExitStack

    with tile.TileContext(nc) as tc, ExitStack() as ctx:
        inp = ctx.enter_context(tc.tile_pool(name="inp", bufs=4))
        dve_scr = ctx.enter_context(tc.tile_pool(name="dve_scr", bufs=2))
        act_scr = ctx.enter_context(tc.tile_pool(name="act_scr", bufs=2))
        accp = ctx.enter_context(
            tc.tile_pool(name="acc", bufs=N_ACCS + 2))
        outp = ctx.enter_context(tc.tile_pool(name="outp", bufs=1))

        for r in range(repeats):
            accs = {c: [] for c in range(C)}
            for c in range(C):
                off = 0
                for FS in SPLITS[c]:
                    xt = inp.tile([P, 2 * FS], mybir.dt.float32, tag="xt")
                    nc.sync.dma_start(out=xt[:, 0:FS],
                                      in_=x_d[c, :, off:off + FS])
                    nc.sync.dma_start(out=xt[:, FS:2 * FS],
                                      in_=t_d[c, :, off:off + FS])
                    off += FS

                    acc_num = accp.tile([P, 1], mybir.dt.float32,
                                        tag="acc_num")
                    acc_den = accp.tile([P, 1], mybir.dt.float32,
                                        tag="acc_den")

                    prod = dve_scr.tile([P, FS], mybir.dt.float32, tag="prod")
                    nc.vector.scalar_tensor_tensor(
                        out=prod[:], in0=xt[:, 0:FS], scalar=1.0,
                        in1=xt[:, FS:2 * FS],
                        op0=mybir.AluOpType.mult, op1=mybir.AluOpType.mult,
                        accum_out=acc_num[:],
                    )
                    sq = act_scr.tile([P, 2 * FS], mybir.dt.float32, tag="sq")
                    nc.scalar.activation(
                        out=sq[:], in_=xt[:],
                        func=mybir.ActivationFunctionType.Square,
                        accum_out=acc_den[:],
                    )
                    accs[c].append((acc_num, acc_den))

            if r == repeats - 1:
                gather = outp.tile([P, 2 * C], mybir.dt.float32)
                for c in range(C):
                    (num0, den0) = accs[c][0]
                    for (num, den) in accs[c][1:]:
                        nc.vector.tensor_add(num0[:], num0[:], num[:])
                        nc.vector.tensor_add(den0[:], den0[:], den[:])
                    nc.vector.tensor_copy(gather[:, c:c + 1], num0[:])
                    nc.vector.tensor_copy(gather[:, C + c:C + c + 1], den0[:])
                nc.sync.dma_start(out=out_d[:], in_=gather[:])


def _shard_inputs(input, target):
    """Rearrange full [16,3,512,512] arrays into 8 per-core channel-major
    [3, 128, 4096] float32 arrays."""
    in_maps = []
    for i in range(N_CORES):
        maps = {}
        for name, arr in (("x", input), ("t", target)):
            shard = arr[i * B_PER_CORE:(i + 1) * B_PER_CORE]  # [2,3,512,512]
            shard = np.ascontiguousarray(
                np.transpose(shard, (1, 0, 2, 3))
            ).reshape(C, P, F)
            maps[name] = shard
        in_maps.append(maps)
    return in_maps


def _finalize(stats_per_core):
    """stats_per_core: list of [128, 6] arrays -> scalar dice loss."""
    total = np.zeros(2 * C, dtype=np.float64)
    for s in stats_per_core:
        total += s.astype(np.float64).sum(axis=0)
    num = 2.0 * total[:C]
    den = total[C:]
    dice = (num + EPS) / (den + EPS)
    return np.array(1.0 - dice.mean(), dtype=np.float32)


_CACHED = {}


def _get_program():
    if "nc" not in _CACHED:
        _CACHED["nc"] = build_dice_program(repeats=1)
    return _CACHED["nc"]


def kernel(input: np.ndarray, target: np.ndarray) -> np.ndarray:
    input = np.asarray(input, dtype=np.float32)
    target = np.asarray(target, dtype=np.float32)
    assert input.shape == (B, C, H, W) and target.shape == (B, C, H, W)

    nc = _get_program()
    in_maps = _shard_inputs(input, target)
    res = run_bass_kernel_spmd(nc, in_maps, core_ids=list(range(N_CORES)))
    return _finalize([res.results[i]["stats"] for i in range(N_CORES)])


# revision 9
# speedup vs baseline: 1.3846x; 1.1386x over previous
"""Trainium2 Bass kernel for nn_DiceLoss (soft dice, binary task, average='batch',
channel_reduction='mean').

    num_c = 2 * sum_{b,n} x[b,c,n] * t[b,c,n]
    den_c = sum x^2 + sum t^2
    out   = 1 - mean_c (num_c + eps) / (den_c + eps)

Strategy (data-parallel over batch, 8 NeuronCores):
  - core i gets batches [2i, 2i+2) of both tensors, rearranged host-side to
    channel-major [3, 128, 4096] so each channel is contiguous
  - per channel, chunked DMA pairs load the x and t halves of a combined
    [128, 2*FS] SBUF tile (1 MiB per DMA, final channel tapered so the last
    ScalarE op on the critical tail is small); VectorE
    computes sum(x*t) per partition via a fused scalar_tensor_tensor
    (mult+mult, accum_out); ScalarE computes sum(x^2)+sum(t^2) per partition
    via one activation(Square, accum_out) over the combined tile (the dice
    denominator only needs the sum, so x^2 and t^2 need not be separated)
  - per-core per-partition partials ([128, 6] per core) are summed on host and
    fed through the (tiny) dice formula in float64

The kernel is memory-bound: 12.6 MiB HBM reads per core (~35 us at the
~358 GB/s per-core HBM limit); VectorE (~13 us) and ScalarE (~21 us) hide
under the DMA.  TimelineSim (production cost model): ~42 us/core.

Self-contained: hardcodes shapes from the problem spec
(input/target: [16, 3, 512, 512] float32).
"""

import numpy as np

import concourse.bass as bass
import concourse.tile as tile
from concourse import mybir
from concourse.bass_utils import run_bass_kernel_spmd

EPS = 1e-05

B, C, H, W = 16, 3, 512, 512
N_CORES = 8
B_PER_CORE = B // N_CORES            # 2
SPATIAL = H * W                      # 262144
CH = B_PER_CORE * SPATIAL            # 524288 elems per channel per core
P = 128
F = CH // P                          # 4096
# chunk schedule (free-dim columns per chunk, per channel): 1 MiB DMAs except
# a tapered final channel so the last ScalarE op on the critical tail is small
SPLITS = [[2048, 2048], [2048, 2048], [2048, 1024, 1024]]
N_ACCS = sum(len(s) for s in SPLITS)


# --- walrus workaround: this container's neuronxcc rejects instructions with
# more than one sync-wait; hoist excess waits onto same-engine NOPs. ---
_uid = [0]


def _split_multi_waits(nc):
    for fn in nc.m.functions:
        for bb in fn.blocks:
            insts = bb.instructions
            out = []
            changed = False
            for inst in insts:
                si = inst.sync_info
                waits = list(si.on_wait) if si is not None and si.on_wait else []
                if len(waits) > 1:
                    changed = True
                    for w in waits[:-1]:
                        _uid[0] += 1
                        out.append(mybir.InstNoOp(
                            name=f"waitsplit_{_uid[0]}",
                            engine=inst.engine,
                            bass_nofuse=True,
                            sync_info=mybir.SyncInfo(on_wait=[w], on_update=[]),
                        ))
                    si.on_wait = [waits[-1]]
                out.append(inst)
            if changed:
                bb.instructions = out


def _trimmed_drain_and_barrier(self, tick_clock, wait_clock):
    """Tile kernel tail without the trailing all-engine barrier.

    Default tail: drain(+waits) / barrier / semaphore clear / barrier.  After
    the first barrier every engine is past its last real instruction, and only
    GpSimd runs the dma_reset+sem_clear; the runtime does not report the NEFF
    done until GpSimd's queue (including the clear) retires, so the trailing
    barrier only adds EVSEM-butterfly latency (~us on silicon) and is dropped.
    (The drain's multi-wait list is split by _split_multi_waits afterwards.)
    """
    from concourse.tile import ScopedClock

    nc = self.nc
    drain_inst = nc.sync.drain()
    wait_clock.add_sem_waits(
        drain_inst.ins, ScopedClock({None: tick_clock.global_clock})
    )
    nc.all_engine_barrier()
    assert self.sems is not None
    popped = nc._tile_sem_poison_stack.pop()
    assert popped is self._sem_poison
    nc.clear_and_free_semaphores(list(self.sems.allocated().values()))


def build_dice_program(repeats: int = 1):
    """Bass program: inputs x,t [3, 128, 4096] f32; output stats [128, 6] f32.

    stats[:, c]   = per-partition sum of x*t for channel c
    stats[:, 3+c] = per-partition sum of x^2 + t^2 for channel c

    repeats > 1 re-runs the whole body (for marginal-time benchmarking);
    only the last repeat's stats land in the output.
    """
    from contextlib import ExitStack

    nc = bass.Bass("TRN2", target_bir_lowering=False, debug=False,
                   num_devices=N_CORES)
    x_d = nc.dram_tensor("x", [C, P, F], mybir.dt.float32,
                         kind="ExternalInput").ap()
    t_d = nc.dram_tensor("t", [C, P, F], mybir.dt.float32,
                         kind="ExternalInput").ap()
    out_d = nc.dram_tensor("stats", [P, 2 * C], mybir.dt.float32,
                           kind="ExternalOutput").ap()

    orig_tail = tile.TileContext._drain_and_barrier
    tile.TileContext._drain_and_barrier = _trimmed_drain_and_barrier
    try:
        _build_body(nc, x_d, t_d, out_d, repeats)
    finally:
        tile.TileContext._drain_and_barrier = orig_tail

    _split_multi_waits(nc)
    return nc


def _build_body(nc, x_d, t_d, out_d, repeats):
    from contextlib import ExitStack

    with tile.TileContext(nc) as tc, ExitStack() as ctx:
        inp = ctx.enter_context(tc.tile_pool(name="inp", bufs=4))
        dve_scr = ctx.enter_context(tc.tile_pool(name="dve_scr", bufs=2))
        act_scr = ctx.enter_context(tc.tile_pool(name="act_scr", bufs=2))
        accp = ctx.enter_context(
            tc.tile_pool(name="acc", bufs=2 * N_ACCS + 2))
        outp = ctx.enter_context(tc.tile_pool(name="outp", bufs=1))

        for r in range(repeats):
            accs = {c: [] for c in range(C)}
            for c in range(C):
                off = 0
                for FS in SPLITS[c]:
                    xt = inp.tile([P, 2 * FS], mybir.dt.float32, tag="xt")
                    nc.sync.dma_start(out=xt[:, 0:FS],
                                      in_=x_d[c, :, off:off + FS])
                    nc.sync.dma_start(out=xt[:, FS:2 * FS],
                                      in_=t_d[c, :, off:off + FS])
                    off += FS

                    acc_num = accp.tile([P, 1], mybir.dt.float32,
                                        tag="acc_num")
                    acc_dx = accp.tile([P, 1], mybir.dt.float32,
                                       tag="acc_dx")
                    acc_dt = accp.tile([P, 1], mybir.dt.float32,
                                       tag="acc_dt")

                    prod = dve_scr.tile([P, FS], mybir.dt.float32, tag="prod")
                    nc.vector.scalar_tensor_tensor(
                        out=prod[:], in0=xt[:, 0:FS], scalar=1.0,
                        in1=xt[:, FS:2 * FS],
                        op0=mybir.AluOpType.mult, op1=mybir.AluOpType.mult,
                        accum_out=acc_num[:],
                    )
                    sqx = act_scr.tile([P, FS], mybir.dt.float32, tag="sqx")
                    nc.scalar.activation(
                        out=sqx[:], in_=xt[:, 0:FS],
                        func=mybir.ActivationFunctionType.Square,
                        accum_out=acc_dx[:],
                    )
                    sqt = act_scr.tile([P, FS], mybir.dt.float32, tag="sqt")
                    nc.scalar.activation(
                        out=sqt[:], in_=xt[:, FS:2 * FS],
                        func=mybir.ActivationFunctionType.Square,
                        accum_out=acc_dt[:],
                    )
                    accs[c].append((acc_num, acc_dx, acc_dt))

            if r == repeats - 1:
                gather = outp.tile([P, 2 * C], mybir.dt.float32)
                for c in range(C):
                    num0 = accs[c][0][0]
                    for entry in accs[c][1:]:
                        nc.vector.tensor_add(num0[:], num0[:], entry[0][:])
                    dens = [a for entry in accs[c] for a in entry[1:]]
                    den0 = dens[0]
                    for d in dens[1:]:
                        nc.vector.tensor_add(den0[:], den0[:], d[:])
                    nc.vector.tensor_copy(gather[:, c:c + 1], num0[:])
                    nc.vector.tensor_copy(gather[:, C + c:C + c + 1], den0[:])
                nc.sync.dma_start(out=out_d[:], in_=gather[:])


def _shard_inputs(input, target):
    """Rearrange full [16,3,512,512] arrays into 8 per-core channel-major
    [3, 128, 4096] float32 arrays."""
    in_maps = []
    for i in range(N_CORES):
        maps = {}
        for name, arr in (("x", input), ("t", target)):
            shard = arr[i * B_PER_CORE:(i + 1) * B_PER_CORE]  # [2,3,512,512]
            shard = np.ascontiguousarray(
                np.transpose(shard, (1, 0, 2, 3))
            ).reshape(C, P, F)
            maps[name] = shard
        in_maps.append(maps)
    return in_maps


def _finalize(stats_per_core):
    """stats_per_core: list of [128, 6] arrays -> scalar dice loss."""
    total = np.zeros(2 * C, dtype=np.float64)
    for s in stats_per_core:
        total += s.astype(np.float64).sum(axis=0)
    num = 2.0 * total[:C]
    den = total[C:]
    dice = (num + EPS) / (den + EPS)
    return np.array(1.0 - dice.mean(), dtype=np.float32)


_CACHED = {}


def _get_program():
    if "nc" not in _CACHED:
        _CACHED["nc"] = build_dice_program(repeats=1)
    return _CACHED["nc"]


def kernel(input: np.ndarray, target: np.ndarray) -> np.ndarray:
    input = np.asarray(input, dtype=np.float32)
    target = np.asarray(target, dtype=np.float32)
    assert input.shape == (B, C, H, W) and target.shape == (B, C, H, W)

    nc = _get_program()
    in_maps = _shard_inputs(input, target)
    res = run_bass_kernel_spmd(nc, in_maps, core_ids=list(range(N_CORES)))
    return _finalize([res.results[i]["stats"] for i in range(N_CORES)])
